# revision 20
# baseline (speedup 1.0000x reference)
# Multi-head causal attention (B=1, T=4096, D=1024, H=16) on 8 TRN2 NeuronCores.
#
# Sharding: tensor-parallel over heads. Core n computes head channels
# [128n, 128n+128) (= heads 2n, 2n+1), runs the full causal attention for its
# two heads, and produces a full-shape partial output
#   y_n = attn_out[:, ch_n] @ Wo[:, ch_n].T        (4096, 1024)
# The host sums the 8 partials (row-sharded Wo contraction) — no collectives.
#
# Device-side layout (per core):
#   xT   [128, 8, T]   x transposed + partition-tiled on the host; contraction
#                      (d_model) on partitions.
#   QT/KT [128, 2, T]  head channels on partitions (h0: 0-63, h1: 64-127), bf16
#   scoresT[j, i]      keys on partitions, queries on free dim.
#
# v2 changes over the 244us baseline:
#  * PV pair packed by COLUMN TILING: per key tile, head0's PV (M=64) lands on
#    psum partitions 0:64 and head1's on 64:128 of ONE [128,512] bank — both
#    matmuls stream concurrently in one ~512-cycle window (measured 216ns/pair
#    vs 432 serial). The V' ones-column is gone (it forced M=65 > 64).
#  * Rowsums via dedicated M=1 ones-matmuls, FOUR per window at col positions
#    0/32/64/96 (two key tiles x two heads), accumulated in a [128,512] psum
#    bank on rows {0,32,64,96}; measured ~216ns per 4-way group. The per-head
#    combine (even+odd rows) is folded into the rowsum->channels broadcast as
#    a K=128 0/1-selector matmul (col-tiled pair, one window per chunk).
#  * Causal partial-N: the 3 boundary key tiles per chunk compute only query
#    columns >= off (N = 512-off) in scores, exp, PV and rowsums; no memsets.
#  * exp split across ScalarE AND VectorE: every DVE_EVERY-th full tile runs
#    on the DVE as z = sc*A (tensor_scalar_mul, frees the psum bank after one
#    pass) followed by a custom 8-stage DVE op (EXP_FIN2) that builds the bf16
#    BIT PATTERN of kappa*exp(sc/8) arithmetically (magic-number rounding + a
#    shifted-parabola mantissa correction, |rel err| < 0.5%) and stores it via
#    the fp32->int16 RNE write conversion into a bf16-viewed tile. The ACT
#    path gets bias=ln(kappa) so both paths share the kappa scale, which then
#    cancels in the softmax normalization.
#  * Normalization: reciprocal_approx_fast on the broadcast [128,512] tile and
#    a single full-tile multiply (pv layout already matches outt layout).
#  * Prologue: fewer HAM warm-up matmuls; tri DMA'd early (chunk 0's exp needs
#    it); exp-table + clock warm-up overlap the ordered weight/x DMAs.
#
# Schedule: flat per-128-key-tile steps per 512-query chunk; at step k the
# kernel emits score matmuls for k+2, the exp for k, PV for k-2 (and the 4-way
# rowsum group when k-2 is odd). The Q/K/V projection of chunk c+1 and the
# tails of chunk c-1 interleave into chunk c's steps via gated generators.
# PSUM banks: 2 rotating 2-bank score tiles (4), one 2-bank aux slot
# (projection/broadcast/output), one PV accumulator (1), one rowsum
# accumulator (1).

import os
import sys

for _p in ("/opt/trn_rl_repo", "/root/.axon_site/_ro/trn_rl_repo"):
    if os.path.isdir(_p) and _p not in sys.path:
        sys.path.insert(0, _p)

import ml_dtypes
import numpy as np


def _ensure_axon_ntff_hook():
    """The agent image's antenv package lacks axon_hooks, which makes
    run_bass_kernel_spmd(trace=True) crash at import under axon. Provide the
    module and register the boot hook so NTFF profiling works."""
    import types

    try:
        import antenv.axon_hooks  # noqa: F401
        return
    except ImportError:
        pass
    try:
        import antenv
    except ImportError:
        return
    mod = types.ModuleType("antenv.axon_hooks")
    mod._hook = None
    mod.set_axon_ntff_profile_hook = lambda h: setattr(mod, "_hook", h)
    mod.get_axon_ntff_profile_hook = lambda: mod._hook
    sys.modules["antenv.axon_hooks"] = mod
    antenv.axon_hooks = mod
    try:
        from trn_agent_boot.trn_boot import _ntff_profile_via_ctypes

        so = "/opt/axon/libaxon_pjrt.so"
        if os.path.exists(so):
            mod._hook = _ntff_profile_via_ctypes(so)
    except Exception:
        pass


_ensure_axon_ntff_hook()

import concourse.bass as bass
import concourse.tile as tile
from concourse import bacc
from concourse import mybir
from concourse.bass_utils import run_bass_kernel_spmd

F32 = mybir.dt.float32
BF16 = mybir.dt.bfloat16
I16 = mybir.dt.int16
EXP = mybir.ActivationFunctionType.Exp
NPBF = ml_dtypes.bfloat16

D = 1024          # d_model
DK = 64           # head dim
CPC = 128         # channels per core (2 heads)
ICH = 512         # query-chunk size
IH = 512          # i-half width (matmul N / psum bank limit)
JT = 128          # key-tile size
NT = D // 128     # d_model tiles

# ---- EXP_FIN2 bit-trick exp constants (fitted for RNE int16 conversion) ----
PRE = 0.125                                  # 1/sqrt(dk)
EXP_A = float(2.0**7 * np.log2(np.e) * PRE)  # z' = sc * A
EXP_MAGIC = 1.5 * 2.0**30                    # rounds z' to multiples of 2^7
EXP_C4 = 1.44                                # parabola vertex (in T units)
EXP_ALPHA = 0.325937 * 2.0**-7               # parabola scale
EXP_KAPPA = 1.41278457                       # both paths emit kappa*exp(x)
EXP_D = (127.0 + 0.414688) * 2.0**7          # exponent bias + phase

# 0 disables the DVE exp offload: the kernel is PE-bound (~83% busy) with
# ScalarE at ~52%, so offloading exp to the DVE only adds queue latency
DVE_EVERY = int(os.environ.get("MHA_DVE_EVERY", "0"))

_NC_CACHE = {}
_FIN2 = None


def _register_exp_fin2():
    """Register the EXP_FIN2 custom DVE op (monkeypatch of dve_ops' registry;
    nothing on disk changes). From z' = sc*A (fp32), produce the int16 bf16
    bit pattern of kappa*exp(sc*PRE):
      U = z'+M; K = U-M; T = z'-K; dm = |T-C4|; P = dm^2 * alpha;
      v = (z'+D) + P; out = int16(v) == bf16 bits."""
    global _FIN2
    if _FIN2 is not None:
        return _FIN2
    from concourse import dve_ops as DO
    from concourse.dve_spec import (
        Spec, Src0, C0, C1, C2, C3, AluOp, Bin, lower, sq, _spill_c3_to_src1,
        _has_src1,
    )
    from concourse.dve_uop import DveOpSpec

    def _ref(in0, in1, s0, s1, imm2):
        f = np.float32
        z = in0.astype(f)
        U = (z + f(s0)).astype(f)
        K = (U - f(s0)).astype(f)
        Tv = (z - K).astype(f)
        c4 = np.asarray(in1, f).reshape(-1, 1)
        dm = np.abs(Tv - c4).astype(f)
        P = ((dm * dm).astype(f) * f(imm2)).astype(f)
        w = (z + f(s1)).astype(f)
        return (w + P).astype(f)

    name = "EXP_FIN2_ANT"
    if name in DO._SUB_OPCODE_FOR_NAME:
        _FIN2 = next(op for op in DO.OPS if op.name == name)
        return _FIN2
    U = Src0 + C0
    K = U - C0
    Tv = Src0 - K
    dm = Bin(AluOp.ABSOLUTE_DIFF, Tv, C3)
    P = sq(dm) * C2
    w = Src0 + C1
    spec = Spec(body=_spill_c3_to_src1(w + P), reference=_ref)
    row = DO._CUSTOM_DVE_ROW_BASE + len(DO.OPS)
    assert row < 0x20
    shas = {}
    for ver in ("v3", "v4"):
        tmp = DveOpSpec(name=name, opcode=row, uops=lower(spec, ver=ver),
                        rd1_en=_has_src1(spec))
        shas[ver] = tmp.sha(ver)
    op = DO.DveOp(name, spec, subdim=False, uops_sha=shas)
    DO.OPS.append(op)
    DO.CUSTOM_DVE_SPECS[name] = spec
    DO._SUB_OPCODE_FOR_NAME[name] = row
    _FIN2 = op
    return op


def build(T):
    """Build the per-core Bass program for sequence length T."""
    fin2 = _register_exp_fin2()
    nc = bacc.Bacc(None, target_bir_lowering=False, debug=False)
    ich = min(ICH, T)
    nch = T // ich

    xT_d = nc.dram_tensor(
        "xT", [T // ICH if T >= ICH else 1, 128, NT, min(ICH, T)], BF16,
        kind="ExternalInput",
    )
    wqT_d = nc.dram_tensor("wqT", [128, NT, CPC], BF16, kind="ExternalInput")
    wkT_d = nc.dram_tensor("wkT", [128, NT, CPC], BF16, kind="ExternalInput")
    wvT_d = nc.dram_tensor("wvT", [128, NT, CPC], BF16, kind="ExternalInput")
    woT_d = nc.dram_tensor("woT", [CPC, D], BF16, kind="ExternalInput")
    tri_d = nc.dram_tensor("tri", [JT, JT], BF16, kind="ExternalInput")
    ident_d = nc.dram_tensor("ident", [128, 128], BF16, kind="ExternalInput")
    y_d = nc.dram_tensor("y", [T, D], BF16, kind="ExternalOutput")

    with tile.TileContext(nc) as tc:
        with (
            tc.tile_pool(name="const", bufs=1) as const,
            tc.tile_pool(name="xtp", bufs=2) as xtp,
            tc.tile_pool(name="vtp", bufs=2) as vtp,
            tc.tile_pool(name="expp", bufs=6) as expp,
            tc.tile_pool(name="zp", bufs=2) as zp,
            tc.tile_pool(name="outp", bufs=2) as outp,
            tc.tile_pool(name="yp", bufs=4) as yp,
            tc.tile_pool(name="psp", bufs=2, space="PSUM") as psp,
            tc.tile_pool(name="pvp", bufs=1, space="PSUM") as pvp,
        ):
            xt_tiles = {}

            def prefetch_xt(c):
                xt_ch = xtp.tile([128, NT, ich], BF16, tag="xt", name="xt_ch")
                nc.sync.dma_start(out=xt_ch, in_=xT_d[c])
                xt_tiles[c] = xt_ch

            # ---- warm the ScalarE exp table while the first DMAs run ----
            warm = const.tile([128, 1], F32)
            nc.vector.memset(warm, 0.0)
            nc.scalar.activation(out=warm, in_=warm, func=EXP)

            # No dummy HAM warm-up: the ~7.6us fixed engine preamble already
            # covers the first weight/x DMA, so the first real projection
            # matmuls do the clock warm-up themselves (a dummy-warm loop only
            # delays them past the DMA landing).

            # ---- constants; DMA trigger order matters: operands of the first
            # projection matmuls and the first exp's tri mask go first ----
            wq_sb = const.tile([128, NT, 128], BF16)
            wk_sb = const.tile([128, NT, 128], BF16)
            wv_sb = const.tile([128, NT, 128], BF16)
            wo_sb = const.tile([128, D], BF16)
            tri_sb = const.tile([JT, JT], BF16)
            id_sb = const.tile([128, 128], BF16)
            c4_sb = const.tile([128, 1], F32)
            lnk_sb = const.tile([128, 1], F32)
            ones1 = const.tile([128, 1], BF16)
            sel_sb = const.tile([128, 2, DK], F32)
            nc.vector.memset(c4_sb, EXP_C4)
            nc.vector.memset(lnk_sb, float(np.log(EXP_KAPPA)))
            nc.vector.memset(ones1, 1.0)
            nc.vector.memset(sel_sb, 0.0)
            nc.vector.memset(sel_sb[0:1, 0, :], 1.0)
            nc.vector.memset(sel_sb[64:65, 0, :], 1.0)
            nc.vector.memset(sel_sb[32:33, 1, :], 1.0)
            nc.vector.memset(sel_sb[96:97, 1, :], 1.0)

            # zero the rowsum psum bank once: its unused rows (1-31, 33-63, ...)
            # are read by the selector matmul's K=128 contraction with 0-weights
            # and must be finite (stale psum bits could decode as NaN/Inf)
            rs_init = pvp.tile([128, ich], F32, tag="rs", name="rs_init")
            nc.vector.memset(rs_init, 0.0)

            xt_ch0 = xtp.tile([128, NT, ich], BF16, tag="xt", name="xt_ch")
            xt_tiles[0] = xt_ch0
            nc.sync.dma_start(out=wq_sb, in_=wqT_d[:, :, :])
            nc.sync.dma_start(out=xt_ch0[:, 0:2, :], in_=xT_d[0, :, 0:2, :])
            nc.sync.dma_start(out=tri_sb, in_=tri_d[:, :])
            nc.sync.dma_start(out=xt_ch0[:, 2:NT, :], in_=xT_d[0, :, 2:NT, :])
            nc.sync.dma_start(out=wk_sb, in_=wkT_d[:, :, :])
            nc.sync.dma_start(out=wv_sb, in_=wvT_d[:, :, :])
            nc.sync.dma_start(out=id_sb, in_=ident_d[:, :])
            if nch > 1:
                prefetch_xt(1)
            nc.sync.dma_start(out=wo_sb, in_=woT_d[:, :])

            qt_sb = const.tile([128, 2, T], BF16)  # [:,0,:]=QT, [:,1,:]=KT
            # V transposed per key tile: [j, jt, 2*DK] (h0 cols 0:64, h1 64:128)
            vp_sb = const.tile([128, T // JT, 2 * DK], BF16)

            dve_ctr = [0]

            def gen_proj(c):
                """Generator emitting the Q/K/V projection for chunk c,
                one PE/DVE op per yield (the x chunk was prefetched).
                NOTE: must be fully emitted before chunk c's attention."""
                i0 = c * ich
                xt_ch = xt_tiles.pop(c)
                qk_ps = psp.tile([128, 2, ich], F32, tag="aux", bufs=1, name="qk_ps")
                for qk, w_sb in ((0, wq_sb), (1, wk_sb)):
                    for t in range(NT):
                        nc.tensor.matmul(
                            out=qk_ps[:, qk, :],
                            lhsT=w_sb[:, t, :],
                            rhs=xt_ch[:, t, :],
                            start=(t == 0),
                            stop=(t == NT - 1),
                        )
                        yield
                # ScalarE (half idle) does the psum->sbuf casts; keeps the DVE
                # queue short and ScE is the faster PSUM reader anyway
                nc.scalar.copy(out=qt_sb[:, :, i0 : i0 + ich], in_=qk_ps)
                yield
                vt_ps = psp.tile([128, 2, ich], F32, tag="aux", bufs=1, name="vt_ps")
                for t in range(NT):
                    nc.tensor.matmul(
                        out=vt_ps[:, 0, :],
                        lhsT=wv_sb[:, t, :],
                        rhs=xt_ch[:, t, :],
                        start=(t == 0),
                        stop=(t == NT - 1),
                    )
                    yield
                vt_sb = vtp.tile([128, ich], BF16, tag="vt", name="vt_sb")
                nc.scalar.copy(out=vt_sb, in_=vt_ps[:, 0, :])
                yield
                vn_ps = psp.tile(
                    [128, ich // 128, 128], BF16, tag="aux", bufs=1, name="vn_ps"
                )
                for sdx in range(ich // 128):
                    nc.tensor.transpose(
                        out=vn_ps[:, sdx, :],
                        in_=vt_sb[:, sdx * 128 : (sdx + 1) * 128],
                        identity=id_sb,
                    )
                    yield
                jt0 = i0 // JT
                nc.vector.tensor_copy(
                    out=vp_sb[:, jt0 : jt0 + ich // 128, :], in_=vn_ps
                )
                yield

            def gen_tail_head(c, pv, rs, out_slot, last=False):
                """Rowsum + raw-output extraction out of the pv/rs psum banks —
                emitted first thing in the next chunk so the banks free up."""
                rs_sb = outp.tile([128, ich], F32, tag="rs", name="rs_sb")
                outt = outp.tile([128, ich], BF16, tag="outt", name="outt")
                if last:
                    nc.scalar.copy(out=rs_sb, in_=rs)
                    nc.scalar.copy(out=outt, in_=pv)
                else:
                    nc.vector.tensor_copy(out=rs_sb, in_=rs)
                    nc.vector.tensor_copy(out=outt, in_=pv)
                out_slot["rs"] = rs_sb
                out_slot["outt"] = outt
                yield
                # rowsum -> 64-channel broadcast, with the even/odd-row combine
                # folded in: K=128 0/1-selector matmuls, col-tiled pair
                bc_ps = psp.tile([128, IH], F32, tag="aux", bufs=1, name="bc_ps")
                for h in range(2):
                    nc.tensor.matmul(
                        out=bc_ps[h * DK : (h + 1) * DK, :],
                        lhsT=sel_sb[:, h, :],
                        rhs=rs_sb,
                        start=True,
                        stop=True,
                    )
                out_slot["bc_ps"] = bc_ps
                yield

            def gen_tail_bc(c, out_slot, last=False):
                """DVE-only normalization: fast approximate reciprocal of the
                broadcast rowsums, then scale the raw attention output."""
                outt, bc_ps = out_slot["outt"], out_slot["bc_ps"]
                bct = outp.tile([128, ich], F32, tag="bct", name="bct")
                nc.vector.reciprocal_approx_fast(out=bct, in_=bc_ps)
                yield
                # per-half muls so the first y matmuls can start early
                for q in (slice(0, ich // 2), slice(ich // 2, ich)):
                    nc.vector.tensor_mul(outt[:, q], outt[:, q], bct[:, q])
                    yield

            def gen_tail_y(c, out_slot, last=False):
                """Output projection, gated late so the y matmuls reach the
                in-order PE queue only after the normalization chain is done."""
                i0 = c * ich
                outt = out_slot["outt"]
                for sidx in range(ich // 128):
                    y_ps = psp.tile(
                        [128, 2, IH], F32,
                        tag="sc" if last else "aux",
                        bufs=2 if last else 1,
                        name="y_ps",
                    )
                    for e in range(2):
                        nc.tensor.matmul(
                            out=y_ps[:, e, :],
                            lhsT=outt[:, sidx * 128 : (sidx + 1) * 128],
                            rhs=wo_sb[:, e * IH : (e + 1) * IH],
                            start=True,
                            stop=True,
                        )
                        yield
                    y_sb = yp.tile([128, D], BF16, tag="y", name="y_sb")
                    nc.vector.tensor_copy(
                        out=y_sb, in_=y_ps.rearrange("p a b -> p (a b)")
                    )
                    yield
                    r0 = i0 + sidx * 128
                    nc.sync.dma_start(out=y_d[r0 : r0 + 128, :], in_=y_sb)
                    yield

            def emit_chunk(c, pending):
                """Attention steps for chunk c, draining `pending` generator
                entries [min_frac, gen, gate_chunk, op_count] into the slack."""
                i0 = c * ich
                njt = (i0 + ich) // JT
                pv = pvp.tile([128, ich], F32, tag="pv", name="pv")
                rs = pvp.tile([128, ich], F32, tag="rs", name="rs")
                nsteps = njt
                sc_tiles = {}
                ex_tiles = {}

                def emit_sc(k):
                    off = k * JT - i0
                    q0 = max(off, 0)
                    sc = psp.tile([128, 2, IH], F32, tag="sc", name="sc")
                    sc_tiles[k] = sc
                    for h in range(2):
                        hp = slice(h * DK, (h + 1) * DK)
                        nc.tensor.matmul(
                            out=sc[:, h, q0:ich],
                            lhsT=qt_sb[hp, 1, k * JT : (k + 1) * JT],
                            rhs=qt_sb[hp, 0, i0 + q0 : i0 + ich],
                            start=True,
                            stop=True,
                        )

                def emit_exp(k):
                    off = k * JT - i0
                    q0 = max(off, 0)
                    sc = sc_tiles.pop(k)
                    ex = expp.tile([128, 2, IH], BF16, tag="ex", name="ex")
                    ex_tiles[k] = ex
                    use_dve = False
                    if off < 0:
                        dve_ctr[0] += 1
                        use_dve = DVE_EVERY > 0 and dve_ctr[0] % DVE_EVERY == 0
                    if use_dve:
                        z = zp.tile([128, 2, IH], F32, tag="z", name="z")
                        nc.vector.tensor_scalar_mul(z, sc, EXP_A)
                        nc.vector._custom_dve(
                            fin2, out=ex.bitcast(I16), in0=z, in1=c4_sb,
                            s0=EXP_MAGIC, s1=EXP_D, imm2=EXP_ALPHA,
                        )
                    else:
                        nc.scalar.activation(
                            out=ex[:, :, q0:ich], in_=sc[:, :, q0:ich],
                            func=EXP, scale=PRE, bias=lnk_sb[:, 0:1],
                        )
                    if off >= 0:  # boundary tile: causal mask on the diagonal
                        for h in range(2):
                            nc.vector.tensor_mul(
                                ex[:, h, off : off + JT],
                                ex[:, h, off : off + JT],
                                tri_sb,
                            )

                def emit_pv(k):
                    off = k * JT - i0
                    q0 = max(off, 0)
                    ex = ex_tiles[k]
                    for h in range(2):
                        nc.tensor.matmul(
                            out=pv[h * DK : (h + 1) * DK, q0:ich],
                            lhsT=vp_sb[:, k, h * DK : (h + 1) * DK],
                            rhs=ex[:, h, q0:ich],
                            start=(k == 0),
                            stop=(k == njt - 1),
                        )

                def emit_rs(k):
                    # 4-way col-tiled rowsum group for key tiles (k-1, k)
                    for j, (kk, h) in enumerate(
                        ((k - 1, 0), (k - 1, 1), (k, 0), (k, 1))
                    ):
                        off = kk * JT - i0
                        q0 = max(off, 0)
                        p = 32 * j
                        nc.tensor.matmul(
                            out=rs[p : p + 1, q0:ich],
                            lhsT=ones1,
                            rhs=ex_tiles[kk][:, h, q0:ich],
                            start=(k == 1),
                            stop=(k == njt - 1),
                            tile_position=(0, p),
                        )
                    ex_tiles.pop(k - 1)
                    ex_tiles.pop(k)

                def drain(budget, frac):
                    budget += 2 if frac > 0.8 else 0
                    if c == nch - 1 and frac > 0.5:
                        budget += 3
                    while budget > 0:
                        eligible = [
                            e for e in pending
                            if e[2] < c or (e[2] == c and e[0] <= frac)
                        ]
                        if not eligible:
                            return
                        progressed = False
                        for e in eligible:
                            if budget <= 0:
                                return
                            try:
                                next(e[1])
                                budget -= 1
                                progressed = True
                            except StopIteration:
                                pending.remove(e)
                        if not progressed:
                            return

                total_ops = sum(e[3] for e in pending if e[2] <= c)
                emit_sc(0)
                # the previous chunk's lagging PV/rowsum emissions land here,
                # AFTER this chunk's first score window
                for fn in prev_trailing:
                    fn()
                for k in range(nsteps):
                    # scores run only ONE step ahead: sc(k+1) reuses the psum
                    # slot of sc(k-1), whose reader exp(k-1) is just finishing
                    # — a 2-step lookahead would park the PE queue head on
                    # exp(k)'s full latency every step (head-of-line block)
                    emit_exp(k)
                    if k - 2 >= 0:
                        emit_pv(k - 2)
                        if (k - 2) % 2 == 1:
                            emit_rs(k - 2)
                    if k + 1 < nsteps:
                        emit_sc(k + 1)
                    # drained ops must come AFTER this step's attention
                    # emissions (PE head-of-line deadlock otherwise)
                    drain(
                        (total_ops + nsteps - 1 - k) // nsteps + 1,
                        (k + 1) / nsteps,
                    )
                trailing = [
                    lambda: emit_pv(njt - 2),
                    lambda: (emit_pv(njt - 1), emit_rs(njt - 1)),
                ]
                return pv, rs, trailing

            # ---- main schedule ----
            g0 = gen_proj(0)
            for _ in range(2 * NT + 1):
                next(g0)

            def _finish_proj0(g=g0):
                for _ in g:
                    pass

            pending = []
            prev_trailing = [_finish_proj0]
            for c in range(nch):
                if c + 2 < nch:
                    prefetch_xt(c + 2)
                if c + 1 < nch:
                    pending.append([0.0, gen_proj(c + 1), c, 31])
                pv, rs, prev_trailing = emit_chunk(c, pending)
                out_slot = {}
                pending.append(
                    [0.0, gen_tail_head(c, pv, rs, out_slot, last=(c == nch - 1)),
                     c + 1, 2]
                )
                pending.append(
                    [0.20, gen_tail_bc(c, out_slot, last=(c == nch - 1)),
                     c + 1, 3]
                )
                ydef = c + 2 if c < nch - 3 else c + 1
                pending.append(
                    [0.55, gen_tail_y(c, out_slot, last=(c == nch - 1)), ydef,
                     16]
                )
            for fn in prev_trailing:
                fn()
            for e in pending:
                for _ in e[1]:
                    pass
    nc.compile()
    return nc


def get_nc(T):
    if T not in _NC_CACHE:
        _NC_CACHE[T] = build(T)
    return _NC_CACHE[T]


TRI = np.triu(np.ones((JT, JT))).astype(NPBF)  # 1 where key j <= query i
IDENT = np.eye(128).astype(NPBF)

LAST_RESULTS = None  # BassKernelResults of the last run (for profiling)


def _tile_dk(w):
    """[D, C] -> [128, D//128, C] partition-tiled so the device DMA is
    contiguous: out[p, t, c] = w[t*128 + p, c]."""
    Dd, C = w.shape
    return np.ascontiguousarray(
        w.reshape(Dd // 128, 128, C).transpose(1, 0, 2)
    )


def make_in_maps(x, Wq, Wk, Wv, Wo, n_cores=8):
    """x: (T, D) fp32. Returns per-core input maps (bf16 operands)."""
    T = x.shape[0]
    ich = min(ICH, T)
    xT = np.ascontiguousarray(
        x.T.astype(NPBF)
        .reshape(NT, 128, T // ich, ich)
        .transpose(2, 1, 0, 3)
    )
    maps = []
    for n in range(n_cores):
        sl = slice(CPC * n, CPC * (n + 1))
        maps.append(
            {
                "xT": xT,
                "wqT": _tile_dk(Wq[sl, :].T.astype(NPBF)),
                "wkT": _tile_dk(Wk[sl, :].T.astype(NPBF)),
                "wvT": _tile_dk(Wv[sl, :].T.astype(NPBF)),
                "woT": np.ascontiguousarray(Wo[:, sl].T).astype(NPBF),
                "tri": TRI,
                "ident": IDENT,
            }
        )
    return maps


def run(x, Wq, Wk, Wv, Wo, T=None, n_cores=8, trace=False):
    global LAST_RESULTS
    T = T if T is not None else x.shape[0]
    nc = get_nc(T)
    in_maps = make_in_maps(x, Wq, Wk, Wv, Wo, n_cores)
    res = run_bass_kernel_spmd(
        nc, in_maps, core_ids=list(range(n_cores)), trace=trace
    )
    LAST_RESULTS = res
    y = np.zeros((T, D), dtype=np.float64)
    for r in res.results:
        y += r["y"].astype(np.float64)
    return y.astype(np.float32)


def kernel(x, Wq, Wk, Wv, Wo):
    x = np.asarray(x, dtype=np.float32)
    B, T, _ = x.shape
    trace = bool(os.environ.get("MHA_TRACE"))
    y = run(
        np.ascontiguousarray(x.reshape(T, D)),
        np.asarray(Wq, np.float32),
        np.asarray(Wk, np.float32),
        np.asarray(Wv, np.float32),
        np.asarray(Wo, np.float32),
        T=T,
        trace=trace,
    )
    if trace and LAST_RESULTS is not None and LAST_RESULTS.exec_time_ns:
        print(f"HW exec time: {LAST_RESULTS.exec_time_ns} ns")
    return y.reshape(B, T, D)
